# revision 40
# baseline (speedup 1.0000x reference)
"""Causal multi-head attention (B=4, S=2048, D=1024, H=16) on 8 TRN2 NeuronCores.

Sharding: DP=4 over batch x TP=2 over heads (8 heads per core).

v2: fully software-pipelined emission. One flat stream of 160 attention
k-tile steps (4 pairs x 4 q-blocks x causal k-tiles); K/Q projections of
pair p+1, the tail of the V projection, and the output projection are
interleaved into the stream as PE filler so the tensor engine never sits
idle while the ACT engine runs the softmax exp. Crossing (diagonal)
tiles are narrowed to their causal width for scores/exp/mask/PV. Score
matmul head-pairs are issued to distinct PE row groups (tile_position)
so they can co-run. Normalization uses an on-chip reciprocal +
gpsimd partition_broadcast (no DRAM bounce). Inputs stream in
512-column chunks ordered by first use.
"""

import sys

sys.path.insert(0, "/opt/trn_rl_repo")

import numpy as np

B = 4
S = 2048
D = 1024
H = 16
HD = 64
TP = 2
DH = D // TP          # 512 head-dims per core (8 heads)
NHL = DH // HD        # 8 local heads
NPAIR = 4             # head pairs per core
NSC = 4               # 512-wide column blocks of S
KCH = D // 128        # 8 contraction tiles for projections
LAG = 2               # scores->PV software pipeline depth (k-tile steps)

_compiled = None


def _build():
    import concourse.bacc as bacc
    import concourse.mybir as mybir
    import concourse.tile as tile

    F32 = mybir.dt.float32
    BF16 = mybir.dt.bfloat16
    EXP = mybir.ActivationFunctionType.Exp

    nc = bacc.Bacc("TRN2", target_bir_lowering=False, debug=False)

    xq = nc.dram_tensor("xq", [D, S], BF16, kind="ExternalInput")
    xk = nc.dram_tensor("xk", [D, S], BF16, kind="ExternalInput")
    xv = nc.dram_tensor("xv", [D, S], BF16, kind="ExternalInput")
    wq = nc.dram_tensor("wq", [D, DH], BF16, kind="ExternalInput")
    wk = nc.dram_tensor("wk", [D, DH], BF16, kind="ExternalInput")
    wv = nc.dram_tensor("wv", [D, DH], BF16, kind="ExternalInput")
    wo = nc.dram_tensor("wo", [DH, D], BF16, kind="ExternalInput")
    bq_c = nc.dram_tensor("bq_c", [128, NPAIR], F32, kind="ExternalInput")
    bk_c = nc.dram_tensor("bk_c", [128, NPAIR], F32, kind="ExternalInput")
    bv_b = nc.dram_tensor("bv_b", [128, DH], F32, kind="ExternalInput")
    out = nc.dram_tensor("out", [S, D], BF16, kind="ExternalOutput")

    with tile.TileContext(nc) as tc:
        from contextlib import ExitStack

        es = ExitStack()
        cst = es.enter_context(tc.tile_pool(name="cst", bufs=1))
        wp = es.enter_context(tc.tile_pool(name="wp", bufs=1))
        xkqp = es.enter_context(tc.tile_pool(name="xkq", bufs=1))
        vnp = es.enter_context(tc.tile_pool(name="vn", bufs=1))
        ktp = es.enter_context(tc.tile_pool(name="kt", bufs=1))
        atpp = es.enter_context(tc.tile_pool(name="atp", bufs=1))
        prp = es.enter_context(tc.tile_pool(name="pr", bufs=3))
        nrmp = es.enter_context(tc.tile_pool(name="nrm", bufs=1))
        obp = None  # created at the pair-1 boundary, after xvp closes
        psS = es.enter_context(tc.tile_pool(name="psS", bufs=2, space="PSUM"))
        psV = es.enter_context(tc.tile_pool(name="psV", bufs=1, space="PSUM"))
        psA = es.enter_context(tc.tile_pool(name="psA", bufs=2, space="PSUM"))
        # entered last so it can be popped (stack order) at the p1 boundary
        xvp_ctx = tc.tile_pool(name="xvp", bufs=1)
        xvp = xvp_ctx.__enter__()

        # ---------------- constants ----------------
        bqs = cst.tile([128, NPAIR], F32, tag="bqs", name="bqs")
        bks = cst.tile([128, NPAIR], F32, tag="bks", name="bks")
        bvb = cst.tile([128, DH], F32, tag="bvb", name="bvb")
        nc.sync.dma_start(out=bqs[:, :], in_=bq_c[:, :])
        nc.sync.dma_start(out=bks[:, :], in_=bk_c[:, :])
        nc.sync.dma_start(out=bvb[:, :], in_=bv_b[:, :])

        # causal mask [128, 512]: mask[x, y] = 1.0 iff y >= x
        mask = cst.tile([128, 512], BF16, tag="mask", name="mask")
        nc.gpsimd.memset(mask[:, :], 1.0)
        nc.gpsimd.affine_select(
            out=mask[:, :],
            in_=mask[:, :],
            compare_op=mybir.AluOpType.is_ge,
            fill=0.0,
            base=0,
            pattern=[[1, 512]],
            channel_multiplier=-1,
        )

        # ---------------- tiles ----------------
        # weights: one [128, 4096] tile per tensor (8 ki-chunks side by
        # side), loaded by a single partition-folded DMA each
        wvb = wp.tile([128, KCH * DH], BF16, tag="wvb", name="wvb")
        wkb = wp.tile([128, KCH * DH], BF16, tag="wkb", name="wkb")
        wqb = wp.tile([128, KCH * DH], BF16, tag="wqb", name="wqb")
        wvt = [wvb[:, DH * k:DH * (k + 1)] for k in range(KCH)]
        wkt = [wkb[:, DH * k:DH * (k + 1)] for k in range(KCH)]
        wqt = [wqb[:, DH * k:DH * (k + 1)] for k in range(KCH)]
        # x activations: one [128, 4096] tile per (tensor, sc-block);
        # ki-chunk k lives at columns [512k, 512k+512)
        xvb = [xvp.tile([128, KCH * 512], BF16, tag=f"xvb{s}",
                        name=f"xvb{s}") for s in range(NSC)]
        xkb = [xkqp.tile([128, KCH * 512], BF16, tag=f"xkb{s}",
                         name=f"xkb{s}") for s in range(NSC)]
        xqb = [xkqp.tile([128, KCH * 512], BF16, tag=f"xqb{s}",
                         name=f"xqb{s}") for s in range(NSC)]
        xvt = [[xvb[s][:, 512 * k:512 * (k + 1)] for s in range(NSC)]
               for k in range(KCH)]
        xkt = [[xkb[s][:, 512 * k:512 * (k + 1)] for s in range(NSC)]
               for k in range(KCH)]
        xqt = [[xqb[s][:, 512 * k:512 * (k + 1)] for s in range(NSC)]
               for k in range(KCH)]
        # V natural [seq 128, 8*(64+1)]: head h cols 65h..65h+63, ones col 65h+64
        VN = [vnp.tile([128, NHL * (HD + 1)], BF16, tag=f"vn{i}", name=f"VN{i}")
              for i in range(16)]
        # K^T / Q^T per (pair, sc): [128 (2 heads x 64 hd), 512 seq]
        KTt = [[ktp.tile([128, 512], BF16, tag=f"kt{p}_{s}", name=f"KT{p}_{s}")
                for s in range(NSC)] for p in range(NPAIR)]
        QTt = [[ktp.tile([128, 512], BF16, tag=f"qt{p}_{s}", name=f"QT{p}_{s}")
                for s in range(NSC)] for p in range(NPAIR)]
        # attention output A^T per (pair, q-block): [128 dh, 512 q]
        ATP = [[atpp.tile([128, 512], BF16, tag=f"atp{p}_{j}",
                          name=f"ATP{p}_{j}")
                for j in range(NSC)] for p in range(NPAIR)]

        ones = cst.tile([128, NHL], F32, tag="ones", name="ones")
        nc.vector.memset(ones[:, :], 1.0)
        for v in VN:
            nc.vector.tensor_copy(v[:, HD::HD + 1], ones[:, :])

        # PE warm-up: dummy matmuls while the first input DMAs stream in.
        # Keeps the HAM activity monitor at full clock and the PE busy
        # through the pipeline-fill window.
        dmy = cst.tile([128, 512], BF16, tag="dmy", name="dmy")
        nc.vector.memset(dmy[:, :], 0.0)
        dps = psA.tile([128, 512], F32, tag="psA", name="warm_")
        for _ in range(16):
            nc.tensor.matmul(dps[:, :], dmy[:, 0:128], dmy[:, :],
                             start=True, stop=True)

        # ---------------- input DMAs, ordered by first use ----------------
        # one batched DMA per tensor / (tensor, sc-block): dram rows are
        # partition-folded so ki-chunk k lands at free offset 512k
        wkr = wk[:, :].rearrange("(a p) c -> p a c", p=128)
        wqr = wq[:, :].rearrange("(a p) c -> p a c", p=128)
        wvr = wv[:, :].rearrange("(a p) c -> p a c", p=128)
        xkr = xk[:, :].rearrange("(a p) s -> p a s", p=128)
        xqr = xq[:, :].rearrange("(a p) s -> p a s", p=128)
        xvr = xv[:, :].rearrange("(a p) s -> p a s", p=128)
        nc.sync.dma_start(
            out=wkb[:, :].rearrange("p (a c) -> p a c", c=DH), in_=wkr)
        nc.sync.dma_start(
            out=wqb[:, :].rearrange("p (a c) -> p a c", c=DH), in_=wqr)
        nc.sync.dma_start(
            out=xkb[0][:, :].rearrange("p (a s) -> p a s", s=512),
            in_=xkr[:, :, 0:512])
        nc.sync.dma_start(
            out=xqb[0][:, :].rearrange("p (a s) -> p a s", s=512),
            in_=xqr[:, :, 0:512])
        nc.sync.dma_start(
            out=wvb[:, :].rearrange("p (a c) -> p a c", c=DH), in_=wvr)
        nc.sync.dma_start(
            out=xvb[0][:, :].rearrange("p (a s) -> p a s", s=512),
            in_=xvr[:, :, 0:512])
        # remaining sc blocks: k/q (interleaved kq-proj needs them) then v
        for s in range(1, NSC):
            c0, c1 = 512 * s, 512 * (s + 1)
            nc.sync.dma_start(
                out=xkb[s][:, :].rearrange("p (a s) -> p a s", s=512),
                in_=xkr[:, :, c0:c1])
            nc.sync.dma_start(
                out=xqb[s][:, :].rearrange("p (a s) -> p a s", s=512),
                in_=xqr[:, :, c0:c1])
            nc.sync.dma_start(
                out=xvb[s][:, :].rearrange("p (a s) -> p a s", s=512),
                in_=xvr[:, :, c0:c1])

        # ---------------- projection / outproj unit emitters ----------------
        def v_unit_mm(st, ki, ps):
            sc, stp = st // 4, st % 4
            nc.tensor.matmul(
                ps[:, :],
                xvt[ki][sc][:, 128 * stp:128 * (stp + 1)],
                wvt[ki][:, :],
                start=(ki == 0),
                stop=(ki == KCH - 1),
            )

        def v_unit_epi(st, ps):
            vdst = VN[st][:, :].rearrange("p (h c) -> p h c", c=HD + 1)[:, :, :HD]
            nc.vector.tensor_add(
                vdst,
                ps[:, :].rearrange("p (h c) -> p h c", c=HD),
                bvb[:, :].rearrange("p (h c) -> p h c", c=HD),
            )

        def kq_unit_mm(wt, xt, p, sc, ki, ps):
            nc.tensor.matmul(
                ps[:, :],
                wt[ki][:, 128 * p:128 * (p + 1)],
                xt[ki][sc][:, :],
                start=(ki == 0),
                stop=(ki == KCH - 1),
            )

        def kq_unit_epi(dest, bias, p, sc, ps):
            nc.vector.tensor_scalar_add(dest[p][sc][:, :], ps[:, :],
                                        bias[:, p:p + 1])

        def make_unit(mm_fn, epi_fn, n_mm):
            """Return a list of closures, each emitting one filler mm; the
            last also emits the unit epilogue. The psA tile is allocated at
            the first mm."""
            box = {}

            def step(i):
                def go():
                    if i == 0:
                        box['ps'] = psA.tile([128, 512], F32, tag="psA",
                                             name="psA_")
                    mm_fn(i, box['ps'])
                    if i == n_mm - 1:
                        epi_fn(box['ps'])
                return go
            return [step(i) for i in range(n_mm)]

        def v_unit(st):
            return make_unit(lambda i, ps: v_unit_mm(st, i, ps),
                             lambda ps: v_unit_epi(st, ps), KCH)

        def kq_unit(wt, xt, dest, bias, p, sc):
            return make_unit(lambda i, ps: kq_unit_mm(wt, xt, p, sc, i, ps),
                             lambda ps: kq_unit_epi(dest, bias, p, sc, ps),
                             KCH)

        wot = []  # filled at p1 boundary
        pf = {}   # staged c0-2 output-projection partials for j-block 3

        def o_unit(qt, n):
            jq, qq = qt // 4, 128 * (qt % 4)

            def mm(c, ps):
                nc.tensor.matmul(
                    ps[:, :],
                    ATP[c][jq][:, qq:qq + 128],
                    wot[c][:, 512 * n:512 * (n + 1)],
                    start=(c == 0),
                    stop=(c == NPAIR - 1),
                )

            def epi(ps):
                ot = obp.tile([128, 512], BF16, tag="ob", name="ob_")
                nc.vector.tensor_copy(ot[:, :], ps[:, :])
                nc.sync.dma_start(
                    out=out[128 * qt:128 * (qt + 1), 512 * n:512 * (n + 1)],
                    in_=ot[:, :])
            return make_unit(mm, epi, NPAIR)

        def o_partial(qt, n):
            # c = 0..2 contributions of the last j-block, staged to SBUF
            # early so only one matmul + add remains after the final pair
            qq = 128 * (qt % 4)

            def mm(c, ps):
                nc.tensor.matmul(
                    ps[:, :],
                    ATP[c][3][:, qq:qq + 128],
                    wot[c][:, 512 * n:512 * (n + 1)],
                    start=(c == 0),
                    stop=(c == 2),
                )

            def epi(ps):
                t_ = pfp.tile([128, 512], BF16, tag=f"pf{qt}_{n}",
                              name=f"pf{qt}_{n}")
                pf[(qt, n)] = t_
                nc.vector.tensor_copy(t_[:, :], ps[:, :])
            return make_unit(mm, epi, 3)

        def o_final(qt, n):
            qq = 128 * (qt % 4)

            def mm(c, ps):
                nc.tensor.matmul(
                    ps[:, :],
                    ATP[3][3][:, qq:qq + 128],
                    wot[3][:, 512 * n:512 * (n + 1)],
                    start=True,
                    stop=True,
                )

            def epi(ps):
                ot = obp.tile([128, 512], BF16, tag="ob", name="ob_")
                nc.vector.tensor_add(ot[:, :], ps[:, :], pf[(qt, n)][:, :])
                nc.sync.dma_start(
                    out=out[128 * qt:128 * (qt + 1), 512 * n:512 * (n + 1)],
                    in_=ot[:, :])
            return make_unit(mm, epi, 1)

        # ---------------- upfront: K/Q proj of pair 0 sc0, V st0-3 --------
        for f in kq_unit(wkt, xkt, KTt, bks, 0, 0):
            f()
        for f in kq_unit(wqt, xqt, QTt, bqs, 0, 0):
            f()
        for st in range(4):
            for f in v_unit(st):
                f()

        # ---------------- flat attention record list ----------------
        # pair 3 runs its q-blocks largest-first so the interleaved output
        # projection unlocks early and the kernel tail is the smallest block
        records = []
        rec_index = {}
        for p in range(NPAIR):
            for j in range(NSC):
                nk = 4 * (j + 1)
                for kidx in range(nk):
                    rec_index[(p, j, kidx)] = len(records)
                    records.append((p, j, kidx, nk))
        total = len(records)  # 160
        p3_start = next(i for i, r in enumerate(records) if r[0] == 3)

        # ---------------- filler stream with deadlines ----------------
        # Tile semantics follow emission (program) order: every unit that
        # writes a tile MUST be emitted before the record that reads it.
        # Each filler = (deadline_step, is_v, [closures]).  At each step,
        # units whose deadline arrived are force-emitted; otherwise units
        # are paced in deadline order to fill PE idle slots.
        from collections import deque
        filler_units = []
        for st in range(4, 16):
            jst = (st + 4) // 4 - 1  # first q-block containing k-tile st
            dl = rec_index[(0, jst, st)] + LAG - 1
            filler_units.append((dl, True, v_unit(st)))
        for p in range(NPAIR):
            for sc in range(NSC):
                if p == 0 and sc == 0:
                    continue  # emitted upfront
                dlq = rec_index[(p, sc, 0)] - 1
                filler_units.append(
                    (dlq, False, kq_unit(wqt, xqt, QTt, bqs, p, sc)))
                dlk = rec_index[(p, sc, 4 * sc)] - 1
                filler_units.append(
                    (dlk, False, kq_unit(wkt, xkt, KTt, bks, p, sc)))
        filler_units.sort(key=lambda u: u[0])
        fillers = deque()
        for dl, is_v, fs in filler_units:
            for f in fs:
                fillers.append((dl, is_v, f))

        prt_info = {}   # t -> (prt tile, off)
        pv_tiles = {}   # (p, j) -> [pv0, pv1]
        epilogues = {}  # due step -> list of closures

        def emit_scores(t):
            p, j, kidx, nk = records[t]
            i_rel = kidx - 4 * j
            off = 128 * i_rel if i_rel > 0 else 0
            w = 512 - off
            sc_k, kk = kidx // 4, kidx % 4
            pss = psS.tile([128, 1024], F32, tag="pss", name="pss_")
            for h in range(2):
                r0 = 64 * h
                nc.tensor.matmul(
                    pss[:, 512 * h + off:512 * (h + 1)],
                    KTt[p][sc_k][r0:r0 + 64, 128 * kk:128 * (kk + 1)],
                    QTt[p][j][r0:r0 + 64, off:512],
                    start=True,
                    stop=True,
                    tile_position=(r0, 0),
                )
            prt = prp.tile([128, 1024], BF16, tag="prt", name="prt_")
            pss3 = pss[:, :].rearrange("p (h y) -> p h y", y=512)[:, :, off:512]
            prt3 = prt[:, :].rearrange("p (h y) -> p h y", y=512)[:, :, off:512]
            nc.scalar.activation(prt3, pss3, EXP, scale=0.125)
            if i_rel >= 0:
                # only the 128-wide diagonal sub-block needs masking;
                # columns beyond it are fully causal-valid
                msl = mask[:, 0:128]
                for h in range(2):
                    sl = prt[:, 512 * h + off:512 * h + off + 128]
                    nc.vector.tensor_mul(sl, sl, msl)
            prt_info[t] = (prt, off)

        def emit_pv(t):
            p, j, kidx, nk = records[t]
            prt, off = prt_info.pop(t)
            if kidx == 0:
                pv_tiles[(p, j)] = [
                    psV.tile([HD + 1, 512], F32, tag=f"pv{h}", name=f"pv{h}_")
                    for h in range(2)]
            pv = pv_tiles[(p, j)]
            for h in range(2):
                hl = 2 * p + h
                nc.tensor.matmul(
                    pv[h][:, off:512],
                    VN[kidx][:, 65 * hl:65 * hl + 65],
                    prt[:, 512 * h + off:512 * (h + 1)],
                    start=(kidx == 0),
                    stop=(kidx == nk - 1),
                )
            if kidx == nk - 1:
                return True
            return False

        def make_epilogue(p, j):
            def go():
                pv = pv_tiles.pop((p, j))
                at = ATP[p][j]
                for h in range(2):
                    nc.vector.tensor_copy(at[64 * h:64 * h + 64, :],
                                          pv[h][:HD, :])
                for h in range(2):
                    dr = nrmp.tile([1, 512], F32, tag="dr", name="dr_")
                    nc.vector.tensor_copy(dr[:, :], pv[h][HD:HD + 1, :])
                    rr = nrmp.tile([1, 512], F32, tag="rr", name="rr_")
                    nc.vector.reciprocal_approx_fast(rr[:, :], dr[:, :])
                    bct = nrmp.tile([128, 512], F32, tag=f"bct{h}",
                                    name=f"bct{h}_")
                    nc.gpsimd.partition_broadcast(bct[:, :], rr[0:1, :])
                    nc.vector.tensor_mul(at[64 * h:64 * h + 64, :],
                                         at[64 * h:64 * h + 64, :],
                                         bct[64 * h:64 * h + 64, :])
                if p == NPAIR - 2 and j == NSC - 1:
                    # ATP[0..2][3] complete: stage the c0-2 partials of the
                    # last j-block's output projection
                    for qt in range(12, 16):
                        for n in range(2):
                            fillers.extend(
                                (10**9, False, f) for f in o_partial(qt, n))
                if p == NPAIR - 1:
                    units = o_final if j == NSC - 1 else o_unit
                    for qt in range(4 * j, 4 * j + 4):
                        for n in range(2):
                            fillers.extend(
                                (10**9, False, f) for f in units(qt, n))
            return go

        # ---------------- main emission loop ----------------
        acc = 0.0
        t = 0
        while (t < total + LAG or t in epilogues or fillers
               or any(d >= t for d in epilogues)):
            if t < total:
                emit_scores(t)
            tp = t - LAG
            if 0 <= tp < total:
                if emit_pv(tp):
                    p, j = records[tp][0], records[tp][1]
                    epilogues.setdefault(t + 1, []).append(make_epilogue(p, j))
            for f in epilogues.pop(t, ()):
                f()
            # wo pool swap at the start of pair 1: drain remaining V-proj
            # fillers (they read xv tiles), close the xv pool, reuse for wo.
            if t == 41:
                rest = deque()
                while fillers:
                    dl, is_v, f = fillers.popleft()
                    if is_v:
                        f()
                    else:
                        rest.append((dl, False, f))
                fillers = rest
                xvp_ctx.__exit__(None, None, None)
                obp = es.enter_context(tc.tile_pool(name="ob", bufs=3))
                pfp = es.enter_context(tc.tile_pool(name="pfp", bufs=1))
                wop = es.enter_context(tc.tile_pool(name="wo", bufs=1))
                wob = wop.tile([128, NPAIR * D], BF16, tag="wob", name="wob")
                nc.sync.dma_start(
                    out=wob[:, :].rearrange("p (a c) -> p a c", c=D),
                    in_=wo[:, :].rearrange("(a p) c -> p a c", p=128))
                for c in range(NPAIR):
                    wot.append(wob[:, D * c:D * (c + 1)])
            # deadline-forced fillers (fillers is deadline-sorted)
            while fillers and fillers[0][0] <= t:
                fillers.popleft()[2]()
            # paced early emission to fill PE idle slots
            if t < p3_start:
                rate = min(len(fillers) / max(p3_start - t, 1), 4.0)
            else:
                # p3: frontload outproj units as soon as they are gated in
                rate = min(float(len(fillers)), 4.0)
            acc += rate
            while acc >= 1.0 and fillers:
                fillers.popleft()[2]()
                acc -= 1.0
            t += 1
        while fillers:
            fillers.popleft()[2]()

        es.close()

    nc.compile()
    return nc


def kernel(query, key, value, Wq, bq, Wk, bk, Wv, bv, Wo, bo, **trace_kwargs):
    from concourse.bass_utils import run_bass_kernel_spmd

    global _compiled
    if _compiled is None:
        _compiled = _build()
    nc = _compiled

    import ml_dtypes

    BF = ml_dtypes.bfloat16
    query = np.asarray(query, np.float32)
    key = np.asarray(key, np.float32)
    value = np.asarray(value, np.float32)
    Wq, Wk, Wv, Wo = (np.asarray(w, np.float32) for w in (Wq, Wk, Wv, Wo))
    bq, bk, bv, bo = (np.asarray(b_, np.float32) for b_ in (bq, bk, bv, bo))

    xqT = [np.ascontiguousarray(query[b].T).astype(BF) for b in range(B)]
    xkT = [np.ascontiguousarray(key[b].T).astype(BF) for b in range(B)]
    xvT = [np.ascontiguousarray(value[b].T).astype(BF) for b in range(B)]
    shard = []
    for tpi in range(TP):
        cs = slice(DH * tpi, DH * (tpi + 1))
        shard.append({
            "wq": np.ascontiguousarray(Wq[:, cs]).astype(BF),
            "wk": np.ascontiguousarray(Wk[:, cs]).astype(BF),
            "wv": np.ascontiguousarray(Wv[:, cs]).astype(BF),
            "wo": np.ascontiguousarray(Wo[cs, :]).astype(BF),
            "bq_c": np.ascontiguousarray(bq[cs].reshape(NPAIR, 128).T),
            "bk_c": np.ascontiguousarray(bk[cs].reshape(NPAIR, 128).T),
            "bv_b": np.ascontiguousarray(np.broadcast_to(bv[cs], (128, DH))),
        })

    in_maps = []
    for c in range(8):
        b, tpi = c // TP, c % TP
        m = {"xq": xqT[b], "xk": xkT[b], "xv": xvT[b]}
        m.update(shard[tpi])
        in_maps.append(m)

    res = run_bass_kernel_spmd(nc, in_maps, core_ids=list(range(8)),
                               **trace_kwargs)
    outp = np.empty((B, S, D), np.float32)
    for b in range(B):
        outp[b] = (res.results[TP * b]["out"].astype(np.float32)
                   + res.results[TP * b + 1]["out"].astype(np.float32) + bo)
    if trace_kwargs:
        kernel.last_results = res
    return outp


# revision 43
# speedup vs baseline: 1.0327x; 1.0327x over previous
"""Causal multi-head attention (B=4, S=2048, D=1024, H=16) on 8 TRN2 NeuronCores.

Sharding: DP=4 over batch x TP=2 over heads (8 heads per core).

v2: fully software-pipelined emission. One flat stream of 160 attention
k-tile steps (4 pairs x 4 q-blocks x causal k-tiles); K/Q projections of
pair p+1, the tail of the V projection, and the output projection are
interleaved into the stream as PE filler so the tensor engine never sits
idle while the ACT engine runs the softmax exp. Crossing (diagonal)
tiles are narrowed to their causal width for scores/exp/mask/PV. Score
matmul head-pairs are issued to distinct PE row groups (tile_position)
so they can co-run. Normalization uses an on-chip reciprocal +
gpsimd partition_broadcast (no DRAM bounce). Inputs stream in
512-column chunks ordered by first use.
"""

import sys

sys.path.insert(0, "/opt/trn_rl_repo")

import numpy as np

B = 4
S = 2048
D = 1024
H = 16
HD = 64
TP = 2
DH = D // TP          # 512 head-dims per core (8 heads)
NHL = DH // HD        # 8 local heads
NPAIR = 4             # head pairs per core
NSC = 4               # 512-wide column blocks of S
KCH = D // 128        # 8 contraction tiles for projections
LAG = 3               # scores->PV software pipeline depth (k-tile steps)

_compiled = None


def _build():
    import concourse.bacc as bacc
    import concourse.mybir as mybir
    import concourse.tile as tile

    F32 = mybir.dt.float32
    BF16 = mybir.dt.bfloat16
    EXP = mybir.ActivationFunctionType.Exp

    nc = bacc.Bacc("TRN2", target_bir_lowering=False, debug=False)

    xq = nc.dram_tensor("xq", [D, S], BF16, kind="ExternalInput")
    xk = nc.dram_tensor("xk", [D, S], BF16, kind="ExternalInput")
    xv = nc.dram_tensor("xv", [D, S], BF16, kind="ExternalInput")
    wq = nc.dram_tensor("wq", [D, DH], BF16, kind="ExternalInput")
    wk = nc.dram_tensor("wk", [D, DH], BF16, kind="ExternalInput")
    wv = nc.dram_tensor("wv", [D, DH], BF16, kind="ExternalInput")
    wo = nc.dram_tensor("wo", [DH, D], BF16, kind="ExternalInput")
    bq_c = nc.dram_tensor("bq_c", [128, NPAIR], F32, kind="ExternalInput")
    bk_c = nc.dram_tensor("bk_c", [128, NPAIR], F32, kind="ExternalInput")
    bv_b = nc.dram_tensor("bv_b", [128, DH], F32, kind="ExternalInput")
    out = nc.dram_tensor("out", [S, D], BF16, kind="ExternalOutput")

    with tile.TileContext(nc) as tc:
        from contextlib import ExitStack

        es = ExitStack()
        cst = es.enter_context(tc.tile_pool(name="cst", bufs=1))
        wp = es.enter_context(tc.tile_pool(name="wp", bufs=1))
        xkqp = es.enter_context(tc.tile_pool(name="xkq", bufs=1))
        vnp = es.enter_context(tc.tile_pool(name="vn", bufs=1))
        ktp = es.enter_context(tc.tile_pool(name="kt", bufs=1))
        atpp = es.enter_context(tc.tile_pool(name="atp", bufs=1))
        prp = es.enter_context(tc.tile_pool(name="pr", bufs=4))
        nrmp = es.enter_context(tc.tile_pool(name="nrm", bufs=1))
        obp = None  # created at the pair-1 boundary, after xvp closes
        psS = es.enter_context(tc.tile_pool(name="psS", bufs=2, space="PSUM"))
        psV = es.enter_context(tc.tile_pool(name="psV", bufs=1, space="PSUM"))
        psA = es.enter_context(tc.tile_pool(name="psA", bufs=2, space="PSUM"))
        # entered last so it can be popped (stack order) at the p1 boundary
        xvp_ctx = tc.tile_pool(name="xvp", bufs=1)
        xvp = xvp_ctx.__enter__()

        # ---------------- constants ----------------
        bqs = cst.tile([128, NPAIR], F32, tag="bqs", name="bqs")
        bks = cst.tile([128, NPAIR], F32, tag="bks", name="bks")
        bvb = cst.tile([128, DH], F32, tag="bvb", name="bvb")
        nc.sync.dma_start(out=bqs[:, :], in_=bq_c[:, :])
        nc.sync.dma_start(out=bks[:, :], in_=bk_c[:, :])
        nc.sync.dma_start(out=bvb[:, :], in_=bv_b[:, :])

        # causal mask [128, 128]: mask[x, y] = 1.0 iff y >= x
        mask = cst.tile([128, 128], BF16, tag="mask", name="mask")
        nc.gpsimd.memset(mask[:, :], 1.0)
        nc.gpsimd.affine_select(
            out=mask[:, :],
            in_=mask[:, :],
            compare_op=mybir.AluOpType.is_ge,
            fill=0.0,
            base=0,
            pattern=[[1, 128]],
            channel_multiplier=-1,
        )

        # ---------------- tiles ----------------
        # weights: one [128, 4096] tile per tensor (8 ki-chunks side by
        # side), loaded by a single partition-folded DMA each
        wvb = wp.tile([128, KCH * DH], BF16, tag="wvb", name="wvb")
        wkb = wp.tile([128, KCH * DH], BF16, tag="wkb", name="wkb")
        wqb = wp.tile([128, KCH * DH], BF16, tag="wqb", name="wqb")
        wvt = [wvb[:, DH * k:DH * (k + 1)] for k in range(KCH)]
        wkt = [wkb[:, DH * k:DH * (k + 1)] for k in range(KCH)]
        wqt = [wqb[:, DH * k:DH * (k + 1)] for k in range(KCH)]
        # x activations: one [128, 4096] tile per (tensor, sc-block);
        # ki-chunk k lives at columns [512k, 512k+512)
        xvb = [xvp.tile([128, KCH * 512], BF16, tag=f"xvb{s}",
                        name=f"xvb{s}") for s in range(NSC)]
        xkb = [xkqp.tile([128, KCH * 512], BF16, tag=f"xkb{s}",
                         name=f"xkb{s}") for s in range(NSC)]
        xqb = [xkqp.tile([128, KCH * 512], BF16, tag=f"xqb{s}",
                         name=f"xqb{s}") for s in range(NSC)]
        xvt = [[xvb[s][:, 512 * k:512 * (k + 1)] for s in range(NSC)]
               for k in range(KCH)]
        xkt = [[xkb[s][:, 512 * k:512 * (k + 1)] for s in range(NSC)]
               for k in range(KCH)]
        xqt = [[xqb[s][:, 512 * k:512 * (k + 1)] for s in range(NSC)]
               for k in range(KCH)]
        # V natural [seq 128, 8*(64+1)]: head h cols 65h..65h+63, ones col 65h+64
        VN = [vnp.tile([128, NHL * (HD + 1)], BF16, tag=f"vn{i}", name=f"VN{i}")
              for i in range(16)]
        # K^T / Q^T per (pair, sc): [128 (2 heads x 64 hd), 512 seq]
        KTt = [[ktp.tile([128, 512], BF16, tag=f"kt{p}_{s}", name=f"KT{p}_{s}")
                for s in range(NSC)] for p in range(NPAIR)]
        QTt = [[ktp.tile([128, 512], BF16, tag=f"qt{p}_{s}", name=f"QT{p}_{s}")
                for s in range(NSC)] for p in range(NPAIR)]
        # attention output A^T per (pair, q-block): [128 dh, 512 q]
        ATP = [[atpp.tile([128, 512], BF16, tag=f"atp{p}_{j}",
                          name=f"ATP{p}_{j}")
                for j in range(NSC)] for p in range(NPAIR)]

        ones = cst.tile([128, NHL], F32, tag="ones", name="ones")
        nc.vector.memset(ones[:, :], 1.0)
        for v in VN:
            nc.vector.tensor_copy(v[:, HD::HD + 1], ones[:, :])

        # PE warm-up: dummy matmuls while the first input DMAs stream in.
        # Keeps the HAM activity monitor at full clock and the PE busy
        # through the pipeline-fill window.
        dmy = cst.tile([128, 512], BF16, tag="dmy", name="dmy")
        nc.vector.memset(dmy[:, :], 0.0)
        dps = psA.tile([128, 512], F32, tag="psA", name="warm_")
        for _ in range(16):
            nc.tensor.matmul(dps[:, :], dmy[:, 0:128], dmy[:, :],
                             start=True, stop=True)

        # ---------------- input DMAs, ordered by first use ----------------
        # one batched DMA per tensor / (tensor, sc-block): dram rows are
        # partition-folded so ki-chunk k lands at free offset 512k
        wkr = wk[:, :].rearrange("(a p) c -> p a c", p=128)
        wqr = wq[:, :].rearrange("(a p) c -> p a c", p=128)
        wvr = wv[:, :].rearrange("(a p) c -> p a c", p=128)
        xkr = xk[:, :].rearrange("(a p) s -> p a s", p=128)
        xqr = xq[:, :].rearrange("(a p) s -> p a s", p=128)
        xvr = xv[:, :].rearrange("(a p) s -> p a s", p=128)
        nc.sync.dma_start(
            out=wkb[:, :].rearrange("p (a c) -> p a c", c=DH), in_=wkr)
        nc.sync.dma_start(
            out=wqb[:, :].rearrange("p (a c) -> p a c", c=DH), in_=wqr)
        nc.sync.dma_start(
            out=xkb[0][:, :].rearrange("p (a s) -> p a s", s=512),
            in_=xkr[:, :, 0:512])
        nc.sync.dma_start(
            out=xqb[0][:, :].rearrange("p (a s) -> p a s", s=512),
            in_=xqr[:, :, 0:512])
        nc.sync.dma_start(
            out=wvb[:, :].rearrange("p (a c) -> p a c", c=DH), in_=wvr)
        nc.sync.dma_start(
            out=xvb[0][:, :].rearrange("p (a s) -> p a s", s=512),
            in_=xvr[:, :, 0:512])
        # remaining sc blocks: k/q (interleaved kq-proj needs them) then v
        for s in range(1, NSC):
            c0, c1 = 512 * s, 512 * (s + 1)
            nc.sync.dma_start(
                out=xkb[s][:, :].rearrange("p (a s) -> p a s", s=512),
                in_=xkr[:, :, c0:c1])
            nc.sync.dma_start(
                out=xqb[s][:, :].rearrange("p (a s) -> p a s", s=512),
                in_=xqr[:, :, c0:c1])
            nc.sync.dma_start(
                out=xvb[s][:, :].rearrange("p (a s) -> p a s", s=512),
                in_=xvr[:, :, c0:c1])

        # ---------------- projection / outproj unit emitters ----------------
        def v_unit_mm(st, ki, ps):
            sc, stp = st // 4, st % 4
            nc.tensor.matmul(
                ps[:, :],
                xvt[ki][sc][:, 128 * stp:128 * (stp + 1)],
                wvt[ki][:, :],
                start=(ki == 0),
                stop=(ki == KCH - 1),
            )

        def v_unit_epi(st, ps):
            vdst = VN[st][:, :].rearrange("p (h c) -> p h c", c=HD + 1)[:, :, :HD]
            nc.vector.tensor_add(
                vdst,
                ps[:, :].rearrange("p (h c) -> p h c", c=HD),
                bvb[:, :].rearrange("p (h c) -> p h c", c=HD),
            )

        def kq_unit_mm(wt, xt, p, sc, ki, ps):
            nc.tensor.matmul(
                ps[:, :],
                wt[ki][:, 128 * p:128 * (p + 1)],
                xt[ki][sc][:, :],
                start=(ki == 0),
                stop=(ki == KCH - 1),
            )

        def kq_unit_epi(dest, bias, p, sc, ps):
            nc.vector.tensor_scalar_add(dest[p][sc][:, :], ps[:, :],
                                        bias[:, p:p + 1])

        def make_unit(mm_fn, epi_fn, n_mm):
            """Return a list of closures, each emitting one filler mm; the
            last also emits the unit epilogue. The psA tile is allocated at
            the first mm."""
            box = {}

            def step(i):
                def go():
                    if i == 0:
                        box['ps'] = psA.tile([128, 512], F32, tag="psA",
                                             name="psA_")
                    mm_fn(i, box['ps'])
                    if i == n_mm - 1:
                        epi_fn(box['ps'])
                return go
            return [step(i) for i in range(n_mm)]

        def v_unit(st):
            return make_unit(lambda i, ps: v_unit_mm(st, i, ps),
                             lambda ps: v_unit_epi(st, ps), KCH)

        def kq_unit(wt, xt, dest, bias, p, sc):
            return make_unit(lambda i, ps: kq_unit_mm(wt, xt, p, sc, i, ps),
                             lambda ps: kq_unit_epi(dest, bias, p, sc, ps),
                             KCH)

        wot = []  # filled at p1 boundary
        pf = {}   # staged c0-2 output-projection partials for j-block 3

        def o_unit(qt, n):
            jq, qq = qt // 4, 128 * (qt % 4)

            def mm(c, ps):
                nc.tensor.matmul(
                    ps[:, :],
                    ATP[c][jq][:, qq:qq + 128],
                    wot[c][:, 512 * n:512 * (n + 1)],
                    start=(c == 0),
                    stop=(c == NPAIR - 1),
                )

            def epi(ps):
                ot = obp.tile([128, 512], BF16, tag="ob", name="ob_")
                nc.vector.tensor_copy(ot[:, :], ps[:, :])
                nc.sync.dma_start(
                    out=out[128 * qt:128 * (qt + 1), 512 * n:512 * (n + 1)],
                    in_=ot[:, :])
            return make_unit(mm, epi, NPAIR)

        def o_partial(qt, n):
            # c = 0..2 contributions of the last j-block, staged to SBUF
            # early so only one matmul + add remains after the final pair
            qq = 128 * (qt % 4)

            def mm(c, ps):
                nc.tensor.matmul(
                    ps[:, :],
                    ATP[c][3][:, qq:qq + 128],
                    wot[c][:, 512 * n:512 * (n + 1)],
                    start=(c == 0),
                    stop=(c == 2),
                )

            def epi(ps):
                t_ = pfp.tile([128, 512], BF16, tag=f"pf{qt}_{n}",
                              name=f"pf{qt}_{n}")
                pf[(qt, n)] = t_
                nc.vector.tensor_copy(t_[:, :], ps[:, :])
            return make_unit(mm, epi, 3)

        def o_final(qt, n):
            qq = 128 * (qt % 4)

            def mm(c, ps):
                nc.tensor.matmul(
                    ps[:, :],
                    ATP[3][3][:, qq:qq + 128],
                    wot[3][:, 512 * n:512 * (n + 1)],
                    start=True,
                    stop=True,
                )

            def epi(ps):
                ot = obp.tile([128, 512], BF16, tag="ob", name="ob_")
                nc.vector.tensor_add(ot[:, :], ps[:, :], pf[(qt, n)][:, :])
                nc.sync.dma_start(
                    out=out[128 * qt:128 * (qt + 1), 512 * n:512 * (n + 1)],
                    in_=ot[:, :])
            return make_unit(mm, epi, 1)

        # ---------------- upfront: K/Q proj of pair 0 sc0, V st0-3 --------
        for f in kq_unit(wkt, xkt, KTt, bks, 0, 0):
            f()
        for f in kq_unit(wqt, xqt, QTt, bqs, 0, 0):
            f()
        for st in range(4):
            for f in v_unit(st):
                f()

        # ---------------- flat attention record list ----------------
        # pair 3 runs its q-blocks largest-first so the interleaved output
        # projection unlocks early and the kernel tail is the smallest block
        records = []
        rec_index = {}
        for p in range(NPAIR):
            for j in range(NSC):
                nk = 4 * (j + 1)
                for kidx in range(nk):
                    rec_index[(p, j, kidx)] = len(records)
                    records.append((p, j, kidx, nk))
        total = len(records)  # 160
        p3_start = next(i for i, r in enumerate(records) if r[0] == 3)

        # ---------------- filler stream with deadlines ----------------
        # Tile semantics follow emission (program) order: every unit that
        # writes a tile MUST be emitted before the record that reads it.
        # Each filler = (deadline_step, is_v, [closures]).  At each step,
        # units whose deadline arrived are force-emitted; otherwise units
        # are paced in deadline order to fill PE idle slots.
        from collections import deque
        filler_units = []
        for st in range(4, 16):
            jst = (st + 4) // 4 - 1  # first q-block containing k-tile st
            dl = rec_index[(0, jst, st)] + LAG - 1
            filler_units.append((dl, True, v_unit(st)))
        for p in range(NPAIR):
            for sc in range(NSC):
                if p == 0 and sc == 0:
                    continue  # emitted upfront
                dlq = rec_index[(p, sc, 0)] - 1
                filler_units.append(
                    (dlq, False, kq_unit(wqt, xqt, QTt, bqs, p, sc)))
                dlk = rec_index[(p, sc, 4 * sc)] - 1
                filler_units.append(
                    (dlk, False, kq_unit(wkt, xkt, KTt, bks, p, sc)))
        filler_units.sort(key=lambda u: u[0])
        fillers = deque()
        for dl, is_v, fs in filler_units:
            for f in fs:
                fillers.append((dl, is_v, f))

        prt_info = {}   # t -> (prt tile, off)
        pv_tiles = {}   # (p, j) -> [pv0, pv1]
        epilogues = {}  # due step -> list of closures

        def emit_scores(t):
            p, j, kidx, nk = records[t]
            i_rel = kidx - 4 * j
            off = 128 * i_rel if i_rel > 0 else 0
            w = 512 - off
            sc_k, kk = kidx // 4, kidx % 4
            pss = psS.tile([128, 1024], F32, tag="pss", name="pss_")
            for h in range(2):
                r0 = 64 * h
                nc.tensor.matmul(
                    pss[:, 512 * h + off:512 * (h + 1)],
                    KTt[p][sc_k][r0:r0 + 64, 128 * kk:128 * (kk + 1)],
                    QTt[p][j][r0:r0 + 64, off:512],
                    start=True,
                    stop=True,
                    tile_position=(r0, 0),
                )
            prt = prp.tile([128, 1024], BF16, tag="prt", name="prt_")
            pss3 = pss[:, :].rearrange("p (h y) -> p h y", y=512)[:, :, off:512]
            prt3 = prt[:, :].rearrange("p (h y) -> p h y", y=512)[:, :, off:512]
            nc.scalar.activation(prt3, pss3, EXP, scale=0.125)
            if i_rel >= 0:
                # only the 128-wide diagonal sub-block needs masking;
                # columns beyond it are fully causal-valid
                msl = mask[:, 0:128]
                for h in range(2):
                    sl = prt[:, 512 * h + off:512 * h + off + 128]
                    nc.vector.tensor_mul(sl, sl, msl)
            prt_info[t] = (prt, off)

        def emit_pv(t):
            p, j, kidx, nk = records[t]
            prt, off = prt_info.pop(t)
            if kidx == 0:
                pv_tiles[(p, j)] = [
                    psV.tile([HD + 1, 512], F32, tag=f"pv{h}", name=f"pv{h}_")
                    for h in range(2)]
            pv = pv_tiles[(p, j)]
            for h in range(2):
                hl = 2 * p + h
                nc.tensor.matmul(
                    pv[h][:, off:512],
                    VN[kidx][:, 65 * hl:65 * hl + 65],
                    prt[:, 512 * h + off:512 * (h + 1)],
                    start=(kidx == 0),
                    stop=(kidx == nk - 1),
                )
            if kidx == nk - 1:
                return True
            return False

        def make_epilogue(p, j):
            def go():
                pv = pv_tiles.pop((p, j))
                at = ATP[p][j]
                for h in range(2):
                    nc.vector.tensor_copy(at[64 * h:64 * h + 64, :],
                                          pv[h][:HD, :])
                drs, rrs = [], []
                for h in range(2):
                    dr = nrmp.tile([1, 512], F32, tag=f"dr{h}", name=f"dr{h}_")
                    nc.vector.tensor_copy(dr[:, :], pv[h][HD:HD + 1, :])
                    drs.append(dr)
                for h in range(2):
                    rr = nrmp.tile([1, 512], F32, tag=f"rr{h}", name=f"rr{h}_")
                    nc.vector.reciprocal_approx_fast(rr[:, :], drs[h][:, :])
                    rrs.append(rr)
                bcts = []
                for h in range(2):
                    bct = nrmp.tile([128, 512], F32, tag=f"bct{h}",
                                    name=f"bct{h}_")
                    nc.gpsimd.partition_broadcast(bct[:, :], rrs[h][0:1, :])
                    bcts.append(bct)
                for h in range(2):
                    nc.vector.tensor_mul(at[64 * h:64 * h + 64, :],
                                         at[64 * h:64 * h + 64, :],
                                         bcts[h][64 * h:64 * h + 64, :])
                if p == NPAIR - 2 and j == NSC - 1:
                    # ATP[0..2][3] complete: stage the c0-2 partials of the
                    # last j-block's output projection
                    for qt in range(12, 16):
                        for n in range(2):
                            fillers.extend(
                                (10**9, False, f) for f in o_partial(qt, n))
                if p == NPAIR - 1:
                    units = o_final if j == NSC - 1 else o_unit
                    for qt in range(4 * j, 4 * j + 4):
                        for n in range(2):
                            fillers.extend(
                                (10**9, False, f) for f in units(qt, n))
            return go

        # ---------------- main emission loop ----------------
        acc = 0.0
        t = 0
        while (t < total + LAG or t in epilogues or fillers
               or any(d >= t for d in epilogues)):
            if t < total:
                emit_scores(t)
            tp = t - LAG
            if 0 <= tp < total:
                if emit_pv(tp):
                    p, j = records[tp][0], records[tp][1]
                    epilogues.setdefault(t + 1, []).append(make_epilogue(p, j))
            for f in epilogues.pop(t, ()):
                f()
            # wo pool swap at the start of pair 1: drain remaining V-proj
            # fillers (they read xv tiles), close the xv pool, reuse for wo.
            if t == 41:
                rest = deque()
                while fillers:
                    dl, is_v, f = fillers.popleft()
                    if is_v:
                        f()
                    else:
                        rest.append((dl, False, f))
                fillers = rest
                xvp_ctx.__exit__(None, None, None)
                obp = es.enter_context(tc.tile_pool(name="ob", bufs=3))
                pfp = es.enter_context(tc.tile_pool(name="pfp", bufs=1))
                wop = es.enter_context(tc.tile_pool(name="wo", bufs=1))
                wob = wop.tile([128, NPAIR * D], BF16, tag="wob", name="wob")
                nc.sync.dma_start(
                    out=wob[:, :].rearrange("p (a c) -> p a c", c=D),
                    in_=wo[:, :].rearrange("(a p) c -> p a c", p=128))
                for c in range(NPAIR):
                    wot.append(wob[:, D * c:D * (c + 1)])
            # deadline-forced fillers (fillers is deadline-sorted)
            while fillers and fillers[0][0] <= t:
                fillers.popleft()[2]()
            # paced early emission to fill PE idle slots
            if t < p3_start:
                rate = min(len(fillers) / max(p3_start - t, 1), 4.0)
            else:
                # p3: frontload outproj units as soon as they are gated in
                rate = min(float(len(fillers)), 4.0)
            acc += rate
            while acc >= 1.0 and fillers:
                fillers.popleft()[2]()
                acc -= 1.0
            t += 1
        while fillers:
            fillers.popleft()[2]()

        es.close()

    nc.compile()
    return nc


def kernel(query, key, value, Wq, bq, Wk, bk, Wv, bv, Wo, bo, **trace_kwargs):
    from concourse.bass_utils import run_bass_kernel_spmd

    global _compiled
    if _compiled is None:
        _compiled = _build()
    nc = _compiled

    import ml_dtypes

    BF = ml_dtypes.bfloat16
    query = np.asarray(query, np.float32)
    key = np.asarray(key, np.float32)
    value = np.asarray(value, np.float32)
    Wq, Wk, Wv, Wo = (np.asarray(w, np.float32) for w in (Wq, Wk, Wv, Wo))
    bq, bk, bv, bo = (np.asarray(b_, np.float32) for b_ in (bq, bk, bv, bo))

    xqT = [np.ascontiguousarray(query[b].T).astype(BF) for b in range(B)]
    xkT = [np.ascontiguousarray(key[b].T).astype(BF) for b in range(B)]
    xvT = [np.ascontiguousarray(value[b].T).astype(BF) for b in range(B)]
    shard = []
    for tpi in range(TP):
        cs = slice(DH * tpi, DH * (tpi + 1))
        shard.append({
            "wq": np.ascontiguousarray(Wq[:, cs]).astype(BF),
            "wk": np.ascontiguousarray(Wk[:, cs]).astype(BF),
            "wv": np.ascontiguousarray(Wv[:, cs]).astype(BF),
            "wo": np.ascontiguousarray(Wo[cs, :]).astype(BF),
            "bq_c": np.ascontiguousarray(bq[cs].reshape(NPAIR, 128).T),
            "bk_c": np.ascontiguousarray(bk[cs].reshape(NPAIR, 128).T),
            "bv_b": np.ascontiguousarray(np.broadcast_to(bv[cs], (128, DH))),
        })

    in_maps = []
    for c in range(8):
        b, tpi = c // TP, c % TP
        m = {"xq": xqT[b], "xk": xkT[b], "xv": xvT[b]}
        m.update(shard[tpi])
        in_maps.append(m)

    res = run_bass_kernel_spmd(nc, in_maps, core_ids=list(range(8)),
                               **trace_kwargs)
    outp = np.empty((B, S, D), np.float32)
    for b in range(B):
        outp[b] = (res.results[TP * b]["out"].astype(np.float32)
                   + res.results[TP * b + 1]["out"].astype(np.float32) + bo)
    if trace_kwargs:
        kernel.last_results = res
    return outp
